# revision 27
# baseline (speedup 1.0000x reference)
"""Cross-attention block kernel for Trainium2 (8 NeuronCores, SPMD).

Problem: x1 -> Q, x2 -> K,V via a fused qkv linear; per-head attention
softmax(Q K^T / sqrt(hd)) V; output [B, N, D].  B=2, N=2048, D=1024, H=16.

Sharding: batch x heads. Core c owns batch c//4 and heads 4*(c%4) ..
4*(c%4)+3 (256 output dims).  No cross-core communication.

Math restructure vs the straightforward version:
  * K bias dropped on device: the q.bk score term is constant per query
    column, so softmax over keys is invariant to it (exact).
  * V bias + softmax normalization moved to host: the device emits
    unnormalized [AV | rowsum] rows per head (rowsum falls out of the AV
    matmul via a fused ones-column in the V stationary); host computes
    AV/rowsum + bv (exact).  This removes every PE transpose and the
    per-pass reciprocal/scale chatter from the device.
  * V is projected directly into natural [keys, hd] layout (stationary =
    x2^T chunk, moving = Wv slice), so no V transposes either.

Scheduling: the attention inner loop is balanced between PE (4 matmuls
per 128-key chunk) and ACT (one [128,1024] exp per chunk).  All
projection matmuls past the first Q/K quarter are decomposed into
single-matmul work units and injected into the attention chunk stream
(one per chunk as background, plus forced catch-up just before the
first consumer, with K in half-quarter groups so catch-up bursts fit
the 2-chunk scores buffer).  PE stays continuously busy (HAM clock
gate stays open) and the exp stream starts ~22us in (DMA-paced
prologue) and runs gapless from chunk ~16 onward.

Matmul operands are bf16: same single-pass PE rate as float32r, but
2-byte weights enable FWL (fast weight load) + LDWEIGHTS pull-ahead and
row-tile concurrency for the K=64 score pairs, and halve DMA traffic.
Accumulation stays fp32 in PSUM; the softmax/normalization path (exp
input, rowsum, AV accumulate, output) is fp32.
"""

import numpy as np
import ml_dtypes

import concourse.bass as bass
import concourse.mybir as mybir
import concourse.tile as tile
from concourse import bacc
from concourse.bass import ds, ts
from concourse.bass_utils import run_bass_kernel_spmd

B, N, D, H, HD = 2, 2048, 1024, 16, 64
NCORES = 8
GPB = NCORES // B  # head-groups per batch (4)
E = (H // GPB) * HD  # 256 output dims per core (4 heads)
EC = E // 128  # 2 e-chunks per core
DC = D // 128  # 8 d-chunks
SCALE = HD**-0.5

F32 = mybir.dt.float32
BF16 = mybir.dt.bfloat16

NQ = 512  # query block width
NPASS = N // NQ  # 4
NKC = N // 128  # 16 key chunks
KPQ = NKC // NPASS  # 4 key chunks per quarter


def build_nc() -> bass.Bass:
    # Bacc (not plain Bass): its compile() runs move_matmul_waits_to_ldweights
    # + generate_event_semaphores, which split multi-wait matmuls that the
    # TRN2 LDWEIGHTS encoding cannot express.
    nc = bacc.Bacc("TRN2", target_bir_lowering=False, debug=False)

    x1T = nc.dram_tensor("x1t", [D, N], BF16, kind="ExternalInput")
    x2T = nc.dram_tensor("x2t", [D, N], BF16, kind="ExternalInput")
    # weights arrive host-packed as [128, DC, e]: 1:1 with SBUF layout, so
    # the DMA is 2KB-contiguous per partition (8x fewer descriptors than a
    # strided rearrange of [D, E])
    wq0d = nc.dram_tensor("wq0", [128, DC, 128], BF16, kind="ExternalInput")
    wq1d = nc.dram_tensor("wq1", [128, DC, 128], BF16, kind="ExternalInput")
    wk0d = nc.dram_tensor("wk0", [128, DC, 128], BF16, kind="ExternalInput")
    wk1d = nc.dram_tensor("wk1", [128, DC, 128], BF16, kind="ExternalInput")
    wvd = nc.dram_tensor("wvp", [128, DC, E], BF16, kind="ExternalInput")
    bqd = nc.dram_tensor("bq", [E, 1], F32, kind="ExternalInput")  # pre-scaled
    # rows hp*130+0..64 = [head A out | rowsum], +65..129 = head B
    out = nc.dram_tensor("out", [EC * 130, N], F32, kind="ExternalOutput")

    with tile.TileContext(nc) as tc:
        with (
            tc.tile_pool(name="consts", bufs=1) as consts,
            tc.tile_pool(name="x1p", bufs=4) as x1p,
            tc.tile_pool(name="x2p", bufs=4) as x2p,
            tc.tile_pool(name="proj", bufs=1) as proj_pool,
            tc.tile_pool(name="pt", bufs=6) as pt_pool,
            tc.tile_pool(name="osb", bufs=2) as osb_pool,
            # PSUM budget (8 banks): st 2x[128,1024]=4, avA+avB=2,
            # proj accums 2x[128,512]=2.
            tc.tile_pool(name="st", bufs=2, space="PSUM") as st_psum,
            tc.tile_pool(name="av", bufs=1, space="PSUM") as av_psum,
            tc.tile_pool(name="pj", bufs=2, space="PSUM") as pj_psum,
        ):
            ones = consts.tile([128, 1], F32)
            nc.gpsimd.memset(ones, 1.0)

            # ---- DMA issue in first-use order so the prologue's first
            # matmul only waits on wq + the first x1 tile.
            x1_t: dict[int, list] = {}
            x2_t: dict[int, list] = {}

            def dma_x(pool, dram, q, store, nm):
                # one dma_start per quarter: [128, DC, 512] with d-chunks on
                # the free axis (saves ~680ns of SP descriptor-issue per tile)
                xt = pool.tile([128, DC, 512], BF16, tag="x", name=nm)
                nc.sync.dma_start(
                    xt, dram[:, ds(q * 512, 512)].rearrange("(c p) n -> p c n", p=128)
                )
                store[q] = xt

            def dma_x0h(pool, dram, store, nm):
                # quarter 0 in two key-half DMAs: the exp#0 chain only
                # needs the first 256 keys' projection
                halves = []
                for h in range(2):
                    xt = pool.tile([128, DC, 256], BF16, tag="xh", bufs=2, name=nm)
                    nc.sync.dma_start(
                        xt,
                        dram[:, ds(h * 256, 256)].rearrange(
                            "(c p) n -> p c n", p=128
                        ),
                    )
                    halves.append(xt)
                store[0] = halves

            def xsl(store, q, dc):
                return store[q][:, dc, :]

            def x2k(q, dc, half):
                # [128, 256] K-proj moving slice
                t = x2_t[q]
                if isinstance(t, list):
                    return t[half][:, dc, :]
                return t[:, dc, ds(half * 256, 256)]

            def x2v(q, dc, lc):
                # [128, 128] V-proj stationary slice
                t = x2_t[q]
                if isinstance(t, list):
                    return t[lc // 2][:, dc, ds((lc % 2) * 128, 128)]
                return t[:, dc, ds(lc * 128, 128)]

            # wq/wk split per e-chunk half: only the hp0 halves gate exp#0
            def dma_w(dram, nm, ew):
                w = consts.tile([128, DC, ew], BF16, name=nm)
                nc.sync.dma_start(w, dram[:, :, :])
                return w

            wqh = [dma_w(wq0d, "wq0", 128)]
            dma_x(x1p, x1T, 0, x1_t, "x1t0")
            wkh = [dma_w(wk0d, "wk0", 128)]
            dma_x0h(x2p, x2T, x2_t, "x2t0")
            wv = dma_w(wvd, "wv", E)
            wqh.append(dma_w(wq1d, "wq1", 128))
            wkh.append(dma_w(wk1d, "wk1", 128))
            bq = consts.tile([128, EC], F32)
            nc.sync.dma_start(bq, bqd.rearrange("(h p) o -> p (h o)", p=128))
            dma_x(x2p, x2T, 1, x2_t, "x2t1")
            dma_x(x1p, x1T, 1, x1_t, "x1t1")
            dma_x(x2p, x2T, 2, x2_t, "x2t2")
            dma_x(x1p, x1T, 2, x1_t, "x1t2")
            dma_x(x2p, x2T, 3, x2_t, "x2t3")
            dma_x(x1p, x1T, 3, x1_t, "x1t3")

            # per-core SBUF working set (e-chunk dim keeps partitions at 128)
            qTs = proj_pool.tile([128, EC, N], BF16, tag="qts")
            kTs = proj_pool.tile([128, EC, N], BF16, tag="kts")
            # v_sb[:, j, hp] = [v_headA | 1 | v_headB | 1] for key chunk j
            v_sb = proj_pool.tile([128, NKC, EC, 130], BF16, tag="vsb")
            ones_bc = ones[:, None, :].to_broadcast([128, NKC, 1])
            for hp in range(EC):
                nc.vector.tensor_copy(v_sb[:, :, hp, 64:65], ones_bc)
                nc.vector.tensor_copy(v_sb[:, :, hp, 129:130], ones_bc)

            # ---- projection work units (one matmul each; drain rides on
            # the group's last unit) ----
            done: dict[tuple, bool] = {}
            accs: dict[tuple, bass.AP] = {}

            def q_unit(q, hp, dc):
                def f():
                    if dc == 0:
                        accs[("q", q, hp)] = pj_psum.tile(
                            [128, 512], F32, tag="pj", name="qacc"
                        )
                    acc = accs[("q", q, hp)]
                    nc.tensor.matmul(
                        acc,
                        wqh[hp][:, dc, :],
                        xsl(x1_t, q, dc),
                        start=(dc == 0),
                        stop=(dc == DC - 1),
                    )
                    if dc == DC - 1:
                        nc.vector.tensor_scalar(
                            qTs[:, hp, ds(q * 512, 512)],
                            acc[:],
                            SCALE,
                            bq[:, hp : hp + 1],
                            mybir.AluOpType.mult,
                            mybir.AluOpType.add,
                        )
                        done[("Q", q, hp)] = True

                return f

            def k_unit(q, hp, half, dc):
                # half-quarter accumulation [128, 256]: finer guard
                # granularity so pass-0 catch-up bursts fit the st buffer
                def f():
                    if dc == 0:
                        accs[("k", q, hp, half)] = pj_psum.tile(
                            [128, 256], F32, tag="pj", name="kacc"
                        )
                    acc = accs[("k", q, hp, half)]
                    nc.tensor.matmul(
                        acc,
                        wkh[hp][:, dc, :],
                        x2k(q, dc, half),
                        start=(dc == 0),
                        stop=(dc == DC - 1),
                    )
                    if dc == DC - 1:
                        nc.vector.tensor_copy(
                            kTs[:, hp, ds(q * 512 + half * 256, 256)], acc[:]
                        )
                        done[("K", q, hp, half)] = True

                return f

            def v_unit(kc, dc):
                qq, lc = divmod(kc, KPQ)

                def f():
                    if dc == 0:
                        accs[("v", kc)] = pj_psum.tile(
                            [128, 512], F32, tag="pj", name="vacc"
                        )
                    acc = accs[("v", kc)]
                    # natural layout: out[keys, e] accumulated over d-chunks
                    nc.tensor.matmul(
                        acc[:, ds(0, 256)],
                        x2v(qq, dc, lc),
                        wv[:, dc, :],
                        start=(dc == 0),
                        stop=(dc == DC - 1),
                    )
                    if dc == DC - 1:
                        for hp in range(EC):
                            nc.vector.tensor_copy(
                                v_sb[:, kc, hp, 0:64], acc[:, ds(hp * 128, 64)]
                            )
                            nc.vector.tensor_copy(
                                v_sb[:, kc, hp, 65:129],
                                acc[:, ds(hp * 128 + 64, 64)],
                            )
                        done[("V", kc)] = True

                return f

            # ---- prologue: just enough to start pass (p0, hp0) ----
            for dc in range(DC):
                q_unit(0, 0, dc)()
            for dc in range(DC):
                k_unit(0, 0, 0, dc)()

            # ---- background queue, ordered by first-consumer deadline.
            # K(0,1)/Q(0,1) ride along early: they are due at pass-1 start
            # and issue for free inside pass-0's catch-up stalls.
            W: list = []
            W.extend((v_unit(0, dc), 110) for dc in range(DC))
            W.extend((k_unit(0, 0, 1, dc), 110) for dc in range(DC))
            for kc in range(1, KPQ):
                W.extend((v_unit(kc, dc), 110) for dc in range(DC))
            for q in range(1, 4):
                W.extend((k_unit(q, 0, 0, dc), 110) for dc in range(DC))
                W.extend((v_unit(q * KPQ, dc), 110) for dc in range(DC))
                W.extend((k_unit(q, 0, 1, dc), 110) for dc in range(DC))
                for kc in range(q * KPQ + 1, (q + 1) * KPQ):
                    W.extend((v_unit(kc, dc), 110) for dc in range(DC))
            for q in range(1, 4):
                W.extend((q_unit(q, 0, dc), 220) for dc in range(DC))
            for half in range(2):
                W.extend((k_unit(0, 1, half, dc), 110) for dc in range(DC))
            W.extend((q_unit(0, 1, dc), 220) for dc in range(DC))
            for q in range(1, 4):
                for half in range(2):
                    W.extend((k_unit(q, 1, half, dc), 110) for dc in range(DC))
            for q in range(1, 4):
                W.extend((q_unit(q, 1, dc), 220) for dc in range(DC))

            wi = [0]

            def issue_until(key):
                while not done.get(key, False):
                    assert wi[0] < len(W), f"work queue exhausted before {key}"
                    W[wi[0]][0]()
                    wi[0] += 1

            def inject(budget):
                # pop units until ~budget ns of PE time consumed: cheap
                # ap-256 units go 2-per-chunk, ap-512 Q units 1-per-chunk
                while budget > 0 and wi[0] < len(W):
                    fn, cost = W[wi[0]]
                    fn()
                    wi[0] += 1
                    budget -= cost

            # ---- attention passes ----
            # hp-major: all hp0 passes first, so hp1's projections are
            # not needed until pass 4 and drain into later-pass slack
            for hp in range(EC):
                for p in range(NPASS):
                    issue_until(("Q", p, hp))
                    qsl = ds(p * NQ, NQ)
                    avA = av_psum.tile([65, NQ], F32, tag="avA")
                    avB = av_psum.tile([65, NQ], F32, tag="avB")
                    pend = None  # AV emitted one key-chunk behind the scores

                    def av_mms(pt, j, avA=avA, avB=avB, hp=hp):
                        nc.tensor.matmul(
                            avA,
                            v_sb[:, j, hp, 0:65],
                            pt[:, 0:512],
                            start=(j == 0),
                            stop=(j == NKC - 1),
                        )
                        nc.tensor.matmul(
                            avB,
                            v_sb[:, j, hp, 65:130],
                            pt[:, 512:1024],
                            start=(j == 0),
                            stop=(j == NKC - 1),
                        )

                    for j in range(NKC):
                        issue_until(("K", j // KPQ, hp, (j % KPQ) // 2))
                        st = st_psum.tile([128, 1024], F32, tag="st")
                        # scores^T for both heads of e-chunk, row-tiled (K=64)
                        nc.tensor.matmul(
                            st[:, 0:512],
                            kTs[0:64, hp, ts(j, 128)],
                            qTs[0:64, hp, qsl],
                            start=True,
                            stop=True,
                        )
                        nc.tensor.matmul(
                            st[:, 512:1024],
                            kTs[64:128, hp, ts(j, 128)],
                            qTs[64:128, hp, qsl],
                            start=True,
                            stop=True,
                        )
                        pt = pt_pool.tile([128, 1024], BF16, tag="pt")
                        nc.scalar.activation(
                            pt, st, mybir.ActivationFunctionType.Exp
                        )
                        inject(230)
                        if pend is not None:
                            issue_until(("V", pend[1]))
                            av_mms(*pend)
                        pend = (pt, j)
                    issue_until(("V", NKC - 1))
                    av_mms(*pend)

                    # drain unnormalized [out|rowsum] rows straight to DRAM
                    oA = osb_pool.tile([65, NQ], F32, tag="oA")
                    oB = osb_pool.tile([65, NQ], F32, tag="oB")
                    nc.vector.tensor_copy(oA, avA)
                    nc.sync.dma_start(
                        out[ds(hp * 130, 65), ds(p * NQ, NQ)], oA
                    )
                    nc.vector.tensor_copy(oB, avB)
                    nc.sync.dma_start(
                        out[ds(hp * 130 + 65, 65), ds(p * NQ, NQ)], oB
                    )

            assert wi[0] == len(W), f"{len(W) - wi[0]} work units never issued"

    nc.compile()
    return nc


_NC_CACHE = None


def _get_nc():
    global _NC_CACHE
    if _NC_CACHE is None:
        _NC_CACHE = build_nc()
    return _NC_CACHE


_BV = None  # per-core V-bias slices, applied host-side in assemble_out


def make_in_maps(x1, x2, qkv_w, qkv_b):
    global _BV
    x1 = np.asarray(x1, dtype=np.float32)
    x2 = np.asarray(x2, dtype=np.float32)
    qkv_w = np.asarray(qkv_w, dtype=np.float32)
    qkv_b = np.asarray(qkv_b, dtype=np.float32)

    bf16 = ml_dtypes.bfloat16
    x1t = [np.ascontiguousarray(x1[b].T.astype(bf16)) for b in range(B)]
    x2t = [np.ascontiguousarray(x2[b].T.astype(bf16)) for b in range(B)]

    in_maps = []
    bvs = []
    for c in range(NCORES):
        b, g = divmod(c, GPB)
        sl_q = slice(g * E, (g + 1) * E)
        sl_k = slice(D + g * E, D + (g + 1) * E)
        sl_v = slice(2 * D + g * E, 2 * D + (g + 1) * E)
        def pack(sl, e0, ew):
            # [D, ew] slice of W^T -> [128, DC, ew] matching the SBUF tile
            wt = qkv_w[sl].T[:, e0 : e0 + ew].astype(bf16)
            return np.ascontiguousarray(
                wt.reshape(DC, 128, ew).transpose(1, 0, 2)
            )

        in_maps.append(
            {
                "x1t": x1t[b],
                "x2t": x2t[b],
                "wq0": pack(sl_q, 0, 128),
                "wq1": pack(sl_q, 128, 128),
                "wk0": pack(sl_k, 0, 128),
                "wk1": pack(sl_k, 128, 128),
                "wvp": pack(sl_v, 0, E),
                "bq": np.ascontiguousarray(
                    (qkv_b[sl_q] * SCALE).reshape(E, 1)
                ),
            }
        )
        bvs.append(qkv_b[sl_v].copy())
    _BV = bvs
    return in_maps


def assemble_out(results):
    out = np.empty((B, N, D), dtype=np.float32)
    for c, res in enumerate(results):
        b, g = divmod(c, GPB)
        r = res["out"]  # [EC*130, N] unnormalized
        bv = _BV[c]
        for hp in range(EC):
            blk = r[hp * 130 : (hp + 1) * 130]
            for h2 in range(2):
                av = blk[h2 * 65 : h2 * 65 + 64]
                s = blk[h2 * 65 + 64]
                e0 = hp * 128 + h2 * 64
                out[b, :, g * E + e0 : g * E + e0 + 64] = (av / s).T + bv[
                    e0 : e0 + 64
                ]
    return out


def kernel(x1, x2, qkv_w, qkv_b, **run_kwargs):
    nc = _get_nc()
    in_maps = make_in_maps(x1, x2, qkv_w, qkv_b)
    res = run_bass_kernel_spmd(nc, in_maps, list(range(NCORES)), **run_kwargs)
    return assemble_out(res.results)
